# revision 2
# baseline (speedup 1.0000x reference)
"""Causal self-attention (B=2, T=2048, D=1024, 16 heads) on 8 trn2 cores.

Sharding: data-parallel over batch (4 cores per batch element), tensor-parallel
over heads (4 heads per core). Each core computes qkv/attention/proj for its
4 heads and produces a partial [T, D] projection output; the host sums the 4
partials of each batch element.

Host-side prep per core: x[b] transposed to [D, T] (the PE contracts over the
partition dim, so x^T is needed as the streaming operand) and the relevant
w_qkv / w_proj column/row slices, all cast to bf16. The 1/sqrt(d_head) score
scale is folded into w_q and w_k (each gets d_head**-0.25).
"""

import numpy as np
import ml_dtypes

import concourse.bass as bass
import concourse.mybir as mybir
import concourse.tile as tile
from concourse import bacc
from concourse.bass_utils import run_bass_kernel_spmd
from concourse.masks import make_identity, make_upper_triangular

B, T, D = 2, 2048, 1024
NH, DH = 16, 64
HPC = 4  # heads per core
NCORES = 8
KT = D // 128  # 8 contraction chunks for qkv matmuls
NT = T // 128  # 16 sequence chunks

BF16 = mybir.dt.bfloat16
F32 = mybir.dt.float32
EXP = mybir.ActivationFunctionType.Exp

TRACE = False
LAST_RESULTS = None
_NC_CACHE = {}


def _build_program(loop_n=None):
    nc = bacc.Bacc("TRN2", target_bir_lowering=False, debug=False, num_devices=NCORES)
    xT_d = nc.dram_tensor("xT", [D, T], BF16, kind="ExternalInput").ap()
    wqk_d = nc.dram_tensor("wqk", [D, 2 * HPC * DH], BF16, kind="ExternalInput").ap()
    wv_d = nc.dram_tensor("wv", [D, HPC * DH], BF16, kind="ExternalInput").ap()
    wpr_d = nc.dram_tensor("wpr", [HPC * DH, D], BF16, kind="ExternalInput").ap()
    out_d = nc.dram_tensor("out", [T, D], F32, kind="ExternalOutput").ap()

    with tile.TileContext(nc) as tc:
        if loop_n is None:
            _emit(nc, tc, xT_d, wqk_d, wv_d, wpr_d, out_d)
        else:
            hints = (
                mybir.EngineType.PE,
                mybir.EngineType.Activation,
                mybir.EngineType.DVE,
                mybir.EngineType.SP,
                mybir.EngineType.Pool,
            )
            with tc.For_i(0, loop_n, 1, hint_engines=hints):
                _emit(nc, tc, xT_d, wqk_d, wv_d, wpr_d, out_d)
    nc.compile()
    return nc


def _emit(nc, tc, xT_d, wqk_d, wv_d, wpr_d, out_d):
    with (
        tc.tile_pool(name="big", bufs=1) as big,
        tc.tile_pool(name="small", bufs=1) as small,
        tc.tile_pool(name="stage", bufs=3) as stage,
        tc.tile_pool(name="ps_mm", bufs=2, space="PSUM") as ps_mm,
        tc.tile_pool(name="ps_s", bufs=2, space="PSUM") as ps_s,
        tc.tile_pool(name="ps_y", bufs=2, space="PSUM") as ps_y,
    ):
        # ---- load inputs (per-k-chunk DMAs so matmuls can start early) ----
        xT_s = big.tile([128, KT, T], BF16)
        xT_r = xT_d.rearrange("(a p) t -> p a t", p=128)
        for t in range(KT):
            nc.sync.dma_start(out=xT_s[:, t, :], in_=xT_r[:, t, :])
        wqk_s = big.tile([128, KT, 2 * HPC * DH], BF16)
        nc.sync.dma_start(out=wqk_s, in_=wqk_d.rearrange("(a p) n -> p a n", p=128))
        wv_s = big.tile([128, KT, HPC * DH], BF16)
        nc.sync.dma_start(out=wv_s, in_=wv_d.rearrange("(a p) n -> p a n", p=128))
        wpr_s = big.tile([128, 2, D], BF16)
        nc.sync.dma_start(out=wpr_s, in_=wpr_d.rearrange("(a p) n -> p a n", p=128))

        ident = small.tile([128, 128], BF16)
        make_identity(nc, ident)
        # gemask[p, f] = 1.0 where f >= p: the valid (tq >= tk) part of the
        # diagonal 128x128 block of S^T.
        gemask = small.tile([128, 128], BF16)
        make_upper_triangular(nc, gemask, val=1.0, diag=True)

        # q^T / k^T in [d, T] layout: tile jt holds heads 2*jt (parts 0:64)
        # and 2*jt+1 (parts 64:128).
        qT_s = big.tile([128, 2, T], BF16)
        kT_s = big.tile([128, 2, T], BF16)
        # v in natural [tk, d] layout, 65th ones-column per head for rowsums
        v_aug = big.tile([128, NT, 66 * HPC], BF16)
        nc.vector.memset(v_aug, 1.0)
        # exp(S^T) for the current head
        pT = big.tile([128, NT, T], BF16)
        y_all = big.tile([128, NT, HPC * DH], BF16)
        yT_s = big.tile([128, 2, T], BF16)

        # ---- qk^T = wqk.T @ xT  -> [512, T] ----
        for m in range(4):
            for n in range(T // 512):
                ps = ps_mm.tile([128, 512], F32, tag="mm")
                for t in range(KT):
                    nc.tensor.matmul(
                        ps,
                        lhsT=wqk_s[:, t, 128 * m : 128 * (m + 1)],
                        rhs=xT_s[:, t, 512 * n : 512 * (n + 1)],
                        start=(t == 0),
                        stop=(t == KT - 1),
                    )
                dst = qT_s if m < 2 else kT_s
                nc.vector.tensor_copy(dst[:, m % 2, 512 * n : 512 * (n + 1)], ps)

        # ---- v = x @ wv -> [T, 256] natural ----
        for j in range(NT):
            ps = ps_mm.tile([128, HPC * DH], F32, tag="mm")
            for t in range(KT):
                nc.tensor.matmul(
                    ps,
                    lhsT=xT_s[:, t, 128 * j : 128 * (j + 1)],
                    rhs=wv_s[:, t, :],
                    start=(t == 0),
                    stop=(t == KT - 1),
                )
            nc.vector.tensor_copy(
                v_aug[:, j, :].rearrange("p (h c) -> p h c", c=66)[:, :, 0:DH],
                ps.rearrange("p (h c) -> p h c", c=DH),
            )

        # ---- attention, one head at a time ----
        for h in range(HPC):
            jt, base = h // 2, 64 * (h % 2)
            qT_h = qT_s[base : base + 64, jt, :]
            kT_h = kT_s[base : base + 64, jt, :]

            # S^T[tk, tq] = k^T.T @ q^T for the causal (tq >= tk) blocks,
            # exp'd straight out of PSUM into pT (bf16).
            for i in range(NT):
                t0 = 128 * i
                lhsT = kT_h[:, t0 : t0 + 128]
                for c0 in range(t0, T, 1024):
                    w = min(1024, T - c0)
                    ps = ps_s.tile([128, 1024], F32, tag="s")
                    for s0 in range(c0, c0 + w, 512):
                        sw = min(512, c0 + w - s0)
                        nc.tensor.matmul(
                            ps[:, s0 - c0 : s0 - c0 + sw],
                            lhsT=lhsT,
                            rhs=qT_h[:, s0 : s0 + sw],
                            start=True,
                            stop=True,
                        )
                    nc.scalar.activation(pT[:, i, c0 : c0 + w], ps[:, 0:w], EXP)
                # mask the upper-triangular part of the diagonal block
                nc.vector.tensor_mul(
                    pT[:, i, t0 : t0 + 128], pT[:, i, t0 : t0 + 128], gemask
                )

            # y[tq, 0:64] = sum_tk P~[tq, tk] v[tk, :], col 64 = rowsum
            for j in range(NT):
                ps = ps_y.tile([128, 68], F32, tag="y")
                for i in range(j + 1):
                    nc.tensor.matmul(
                        ps[:, 0:65],
                        lhsT=pT[:, i, 128 * j : 128 * (j + 1)],
                        rhs=v_aug[:, i, 66 * h : 66 * h + 65],
                        start=(i == 0),
                        stop=(i == j),
                    )
                rinv = stage.tile([128, 1], F32, tag="rinv")
                nc.vector.reciprocal(rinv, ps[:, DH : DH + 1])
                nc.vector.tensor_scalar_mul(
                    y_all[:, j, DH * h : DH * (h + 1)], ps[:, 0:DH], rinv
                )

        # ---- y^T via PE transpose (proj contracts over d) ----
        for j in range(NT):
            for dm in range(2):
                pst = ps_mm.tile([128, 128], BF16, tag="mm")
                nc.tensor.transpose(pst, y_all[:, j, 128 * dm : 128 * (dm + 1)], ident)
                nc.vector.tensor_copy(yT_s[:, dm, 128 * j : 128 * (j + 1)], pst)

        # ---- out = y @ wpr ----
        for j in range(NT):
            ps = ps_s.tile([128, 1024], F32, tag="s")
            for n in range(2):
                for dm in range(2):
                    nc.tensor.matmul(
                        ps[:, 512 * n : 512 * (n + 1)],
                        lhsT=yT_s[:, dm, 128 * j : 128 * (j + 1)],
                        rhs=wpr_s[:, dm, 512 * n : 512 * (n + 1)],
                        start=(dm == 0),
                        stop=(dm == 1),
                    )
            ost = stage.tile([128, D], F32, tag="ost")
            nc.vector.tensor_copy(ost, ps)
            nc.sync.dma_start(out=out_d[128 * j : 128 * (j + 1), :], in_=ost)


def _get_nc():
    if "nc" not in _NC_CACHE:
        _NC_CACHE["nc"] = _build_program()
    return _NC_CACHE["nc"]


def make_in_maps(x, w_qkv, w_proj):
    bf16 = ml_dtypes.bfloat16
    scale = np.float32(DH**-0.25)
    x = np.asarray(x, dtype=np.float32)
    w_qkv = np.asarray(w_qkv, dtype=np.float32)
    w_proj = np.asarray(w_proj, dtype=np.float32)
    xT_b = [np.ascontiguousarray(x[b].T).astype(bf16) for b in range(B)]
    in_maps = []
    for c in range(NCORES):
        b, g = c // HPC, c % HPC
        cs = slice(g * HPC * DH, (g + 1) * HPC * DH)  # 256 cols of this head group
        wq = w_qkv[:, 0 * D : 1 * D][:, cs] * scale
        wk = w_qkv[:, 1 * D : 2 * D][:, cs] * scale
        in_maps.append(
            {
                "xT": xT_b[b],
                "wqk": np.concatenate([wq, wk], axis=1).astype(bf16),
                "wv": np.ascontiguousarray(w_qkv[:, 2 * D : 3 * D][:, cs]).astype(bf16),
                "wpr": np.ascontiguousarray(w_proj[cs, :]).astype(bf16),
            }
        )
    return in_maps


def kernel(x, w_qkv, w_proj):
    global LAST_RESULTS
    nc = _get_nc()
    in_maps = make_in_maps(x, w_qkv, w_proj)
    res = run_bass_kernel_spmd(nc, in_maps, list(range(NCORES)), trace=TRACE)
    LAST_RESULTS = res
    parts = [np.asarray(res.results[c]["out"], dtype=np.float32) for c in range(NCORES)]
    out = np.stack([sum(parts[b * HPC : (b + 1) * HPC]) for b in range(B)], axis=0)
    return out.astype(np.float32)
